# revision 33
# baseline (speedup 1.0000x reference)
"""Multi-head attention forward, sharded over 8 NeuronCores.

Sharding: batch (2) x head-group (4 groups of 4 heads) = 8 cores.
Each core computes, for its batch b and its 4 heads:
  Q^T/K^T = W^T-slices @ x^T (+bias via per-partition tensor_scalar add),
  V token-major over compacted keys (+bias via a pad-masked bias input
  added during evac -- pad key columns of xTk are zeroed host-side so
  pad keys have V=0 and denominator-weight 0 and drop out of softmax),
  S^T[k,q] = K^T.T@Q^T per k-tile (scores transposed so exp output feeds
  P.V directly), P^T = exp(scale*S^T) with no mask bias,
  ctx_aug^T = [V|1x64]^T.T @ P^T -- the ones block is replicated 64 wide
  so PSUM partitions 64:128 all hold the softmax denominator, i.e. the
  denominator arrives pre-broadcast and normalization is reciprocal +
  multiply reading PSUM directly on DVE,
  out_partial = ctx^T.T @ W_o^T-slice  ->  [2048, 1024] bf16.
Host sums the 4 partials per batch (fp32) and adds out_b.

Schedule: the ACT exp stream (72 x [128,1024] tiles, ~1.04us each) is
the pacing resource; the PE must never micro-idle or the HW HAM
throttle drops the array to half throughput.  Emission is therefore
exp-paced: each S^T tile-pair (the two 64-contraction head matmuls run
CONCURRENTLY on HW via disjoint tile_position row groups, ~220ns) is
topped up with ~1us of dependency-free PE filler.  K/Q/V projections
are double-buffered by rep parity so the NEXT rep's projections float
freely as filler anywhere in the current rep; PV runs as 3-matmul
sub-chains pinned to the slot after its segment's exp; outproj(qc)
lands in slot 2qc+3 (qc-major segment order => both prs' norms for qc
are done by then); the last segment's PV/norm/outproj form a short
dense tail whose evacuations split across DVE and ACT.
"""

import os
import sys

if "/opt/trn_rl_repo" not in sys.path:
    sys.path.insert(0, "/opt/trn_rl_repo")

import numpy as np
import ml_dtypes

import concourse.bass as bass
import concourse.mybir as mybir
from concourse import bacc
from concourse.bass import ts, ds
from concourse.tile import TileContext
from concourse import bass_utils

BF16 = mybir.dt.bfloat16
F32 = mybir.dt.float32
F32R = mybir.dt.float32r
EXP = mybir.ActivationFunctionType.Exp
COPY = mybir.ActivationFunctionType.Copy
MULT = mybir.AluOpType.mult

N_CORES = 8
S = 2048          # sequence length (one batch per core)
HID = 1024
DH = 256          # head dims per core (4 heads x 64)
D = 64
KC = 1280         # compacted+padded key length; runtime-adjusted in kernel()
NKT = KC // 128


def build_program(reps=1):
    ABL = os.environ.get("ABLATE", "")
    nc = bacc.Bacc("TRN2", target_bir_lowering=False, debug=False,
                   num_devices=N_CORES)
    xT = nc.dram_tensor("xT", [HID, S], BF16, kind="ExternalInput").ap()
    xTk = nc.dram_tensor("xTk", [HID, KC], BF16, kind="ExternalInput").ap()
    wqT = nc.dram_tensor("wqT", [HID, DH], BF16, kind="ExternalInput").ap()
    wkT = nc.dram_tensor("wkT", [HID, DH], BF16, kind="ExternalInput").ap()
    wvT = nc.dram_tensor("wvT", [HID, DH], BF16, kind="ExternalInput").ap()
    woT = nc.dram_tensor("woT", [DH, HID], BF16, kind="ExternalInput").ap()
    # per-partition Q/K bias columns: [q pr0, q pr1, k pr0, k pr1]
    qkvbT = nc.dram_tensor("qkvbT", [128, 4], F32,
                           kind="ExternalInput").ap()
    # V bias replicated per key, zeroed on pad keys: [key, kt, 4*64]
    vbias = nc.dram_tensor("vbias", [128, NKT, DH], BF16,
                           kind="ExternalInput").ap()
    # denominator ones block: 1.0 for real keys, 0.0 for pads, x64 wide
    vone64 = nc.dram_tensor("vone64", [128, NKT, 64], BF16,
                            kind="ExternalInput").ap()
    op_dma = os.environ.get("OP_DMA", "sbuf")
    out_dt = F32 if op_dma == "psum" else BF16
    out = nc.dram_tensor("out", [S, HID], out_dt,
                         kind="ExternalOutput").ap()

    with TileContext(nc) as tc:
        with tc.tile_pool(name="const", bufs=1) as cp:
            wq_sb = cp.tile([128, 8, DH], BF16, name="wq_sb")
            wk_sb = cp.tile([128, 8, DH], BF16, name="wk_sb")
            wv_sb = cp.tile([128, 8, DH], BF16, name="wv_sb")
            nc.sync.dma_start(wq_sb, wqT.rearrange("(c p) m -> p c m", p=128))
            nc.sync.dma_start(wk_sb, wkT.rearrange("(c p) m -> p c m", p=128))
            nc.sync.dma_start(wv_sb, wvT.rearrange("(c p) m -> p c m", p=128))
            wo_sb = cp.tile([128, 2, HID], BF16, name="wo_sb")
            nc.sync.dma_start(wo_sb, woT.rearrange("(c p) o -> p c o", p=128))
            qkvbT_sb = cp.tile([128, 4], F32, name="qkvbT_sb")
            nc.sync.dma_start(qkvbT_sb, qkvbT)
            vbias_sb = cp.tile([128, NKT, DH], BF16, name="vbias_sb")
            nc.sync.dma_start(vbias_sb, vbias)

            xt_sb = cp.tile([128, 8, S], BF16, name="xt_sb")
            xt_view = xT.rearrange("(c p) t -> c p t", p=128)
            xtk_sb = cp.tile([128, 8, KC], BF16, name="xtk_sb")
            xtk_view = xTk.rearrange("(c p) t -> c p t", p=128)
            for c in range(8):
                nc.sync.dma_start(xt_sb[:, c, :], xt_view[c])
                nc.sync.dma_start(xtk_sb[:, c, :], xtk_view[c])

            # K/Q/V double-buffered by rep parity: the NEXT rep's
            # projections are emitted as filler anywhere in the current
            # rep with no WAR coupling to this rep's attention reads.
            q_sb = [[cp.tile([128, S], BF16, name=f"q_sb{p}{j}")
                     for j in range(2)] for p in range(2)]
            k_sb = [[cp.tile([128, KC], BF16, name=f"k_sb{p}{j}")
                     for j in range(2)] for p in range(2)]
            # [key-in-tile, kt, head, 64 V dims + 64 denominator ones]
            v_sb = [cp.tile([128, NKT, 4, 128], BF16, name=f"v_sb{p}")
                    for p in range(2)]
            for p in range(2):
                for h in range(4):
                    nc.sync.dma_start(v_sb[p][:, :, h, 64:128], vone64)
            ctxT = [cp.tile([128, S], BF16, name=f"ctxT{j}") for j in range(2)]
            if ABL:                      # keep read-before-write legal
                for p in range(2):
                    for j in range(2):
                        nc.vector.memset(q_sb[p][j], 0.01)
                        nc.vector.memset(k_sb[p][j], 0.01)
                    nc.vector.memset(v_sb[p][:, :, :, 0:64], 0.01)
                for j in range(2):
                    nc.vector.memset(ctxT[j], 0.01)
                sexp_const = cp.tile([128, 1024], F32, name="sexp_const")
                nc.vector.memset(sexp_const, 0.01)
                pt_const = cp.tile([128, 1024], BF16, name="pt_const")
                nc.vector.memset(pt_const, 0.01)

            with tc.tile_pool(name="psS", bufs=2, space="PSUM") as psS, \
                 tc.tile_pool(name="psX", bufs=2, space="PSUM") as psX, \
                 tc.tile_pool(name="shr", bufs=2, space="PSUM") as shr, \
                 tc.tile_pool(name="ptp",
                              bufs=int(os.environ.get("PT_BUFS", "20"))) \
                     as ptp, \
                 tc.tile_pool(name="nrm", bufs=7) as nrm, \
                 tc.tile_pool(name="outp", bufs=3) as outp:

                # PE warmup: ~6us of dummy matmuls that depend only on
                # a local memset, so the array ramps to full clock (HAM
                # K=8/8) while the input DMAs are still landing instead
                # of entering the real stream cold.
                nwarm = int(os.environ.get("WARM", "30"))
                if nwarm:
                    warm = cp.tile([128, 512], BF16, name="warm")
                    nc.vector.memset(warm, 0.5)
                    wps = psS.tile([128, 1024], F32, name="s_ps")
                    for i in range(nwarm):
                        nc.tensor.matmul(wps[:, 0:512],
                                         lhsT=warm[:, 0:128], rhs=warm,
                                         start=(i == 0),
                                         stop=(i == nwarm - 1))

                # ---------- emission helpers ----------
                def kq_jobs():
                    jobs = []
                    off = 0
                    while off < KC:           # K chunks (compact tokens)
                        w = min(512, KC - off)
                        jobs.append(("k", off, w))
                        off += w
                    for n in range(4):        # Q chunks (all tokens)
                        jobs.append(("q", n * 512, 512))
                    return jobs

                IDENT = mybir.ActivationFunctionType.Identity
                kq_ctr = [0]

                def emit_kq_job(par, pr, job, pre=False):
                    """One K/Q projection chunk (8 matmuls + bias evac).
                    Evacuation alternates DVE tensor_scalar / ACT
                    Identity+bias so consecutive shr slots never wait on
                    the same engine."""
                    if "q" in ABL:
                        return
                    kind, off, w = job
                    if kind == "k":
                        src_sb, w_sb = xtk_sb, wk_sb
                        dst, bcol = k_sb[par][pr], 2 + pr
                    else:
                        src_sb, w_sb = xt_sb, wq_sb
                        dst, bcol = q_sb[par][pr], pr
                    if pre:
                        ps = psS.tile([128, 1024], F32, name="s_ps")
                    else:
                        ps = shr.tile([128, 512], F32,
                                      name="shr_ps", tag="shr")
                    for c in range(8):
                        nc.tensor.matmul(
                            ps[:, 0:w],
                            lhsT=w_sb[:, c, ts(pr, 128)],
                            rhs=src_sb[:, c, ds(off, w)],
                            start=(c == 0), stop=(c == 7))
                    kq_ctr[0] += 1
                    kq_evac = os.environ.get("KQ_EVAC", "dve")
                    kqch = int(os.environ.get("KQCH", "1"))
                    if (kq_evac == "act"
                            or (kq_evac == "alt" and kq_ctr[0] % 2 == 0)):
                        nc.scalar.activation(
                            dst[:, ds(off, w)], ps[:, 0:w], IDENT,
                            bias=qkvbT_sb[:, bcol:bcol + 1])
                    else:
                        cw = w // kqch
                        for cc in range(kqch):
                            nc.vector.tensor_scalar_add(
                                dst[:, ds(off + cc * cw, cw)],
                                ps[:, ds(cc * cw, cw)],
                                qkvbT_sb[:, bcol:bcol + 1])

                def emit_v(par, kt):
                    """V projection for one key tile, all 4 heads."""
                    if "v" in ABL:
                        return
                    ps = shr.tile([128, 512], F32, name="shr_ps", tag="shr")
                    for c in range(8):
                        nc.tensor.matmul(ps[:, 0:DH],
                                         lhsT=xtk_sb[:, c, ts(kt, 128)],
                                         rhs=wv_sb[:, c, :],
                                         start=(c == 0), stop=(c == 7))
                    nc.vector.tensor_tensor(
                        out=v_sb[par][:, kt, :, 0:64],
                        in0=ps[:, 0:DH].rearrange("p (h d) -> p h d", h=4),
                        in1=vbias_sb[:, kt, :].rearrange(
                            "p (h d) -> p h d", h=4),
                        op=mybir.AluOpType.add)

                # ---------- attention helpers ----------
                pt_tiles = {}     # (pr, qc) -> list of pt tiles

                def emit_s(par, pr, qc, kt):
                    if "s" not in ABL:
                        s_ps = psS.tile([128, 1024], F32, name="s_ps")
                        for col in range(2):
                            hr = col * 64
                            nc.tensor.matmul(
                                s_ps[:, ts(col, 512)],
                                lhsT=k_sb[par][pr][hr:hr + 64, ts(kt, 128)],
                                rhs=q_sb[par][pr][hr:hr + 64,
                                                  ds(qc * 512, 512)],
                                start=True, stop=True,
                                tile_position=(hr, 0))
                    else:
                        s_ps = sexp_const
                    if "e" not in ABL:
                        pt = ptp.tile([128, 1024], BF16, name="pt")
                        nc.scalar.activation(pt, s_ps, EXP, scale=0.125)
                    else:
                        pt = pt_const
                    pt_tiles.setdefault((pr, qc), []).append(pt)

                ctx_tiles = {}    # (pr, qc) -> (ctxA, ctxB)

                def emit_pv_sub(par, pr, qc, col, klo, khi):
                    """Part of one head's accumulation chain (kt klo:khi)."""
                    if "p" in ABL:
                        return
                    if col == 0 and klo == 0:
                        ctx_tiles[(pr, qc)] = (
                            psX.tile([128, 512], F32, name="ctx_ps"),
                            psX.tile([128, 512], F32, name="ctx_ps"))
                    jctx = ctx_tiles[(pr, qc)][col]
                    pts = pt_tiles[(pr, qc)]
                    for kt in range(klo, khi):
                        nc.tensor.matmul(
                            jctx, lhsT=v_sb[par][:, kt, 2 * pr + col, :],
                            rhs=pts[kt][:, ts(col, 512)],
                            start=(kt == 0), stop=(kt == NKT - 1))

                nstage = int(os.environ.get("NSTAGE", "0"))

                def emit_norm_col(pr, qc, col, chunks=1):
                    if "n" in ABL or "p" in ABL:
                        return
                    jctx = ctx_tiles[(pr, qc)][col]
                    # partitions 64:128 already hold the denominator.
                    hr = col * 64
                    w = 512 // chunks
                    if nstage:
                        # stage through ACT to bf16 SBUF: the DVE recip +
                        # multiply then run in 2-byte 2x/4x perf modes
                        # (~4x cheaper on the coupling engine); bf16
                        # denominator costs ~0.4% rel err (budget 2e-2)
                        csb = nrm.tile([128, 512], BF16, name="csb")
                        nc.scalar.activation(csb, jctx, COPY)
                        for c in range(chunks):
                            recip = nrm.tile([64, 512], BF16, name="recip")
                            with nc.allow_low_precision(reason="recip"):
                                nc.vector.reciprocal(
                                    recip[:, 0:w],
                                    csb[64:128, ds(c * w, w)])
                            nc.vector.tensor_tensor(
                                out=ctxT[pr][hr:hr + 64,
                                             ds(qc * 512 + c * w, w)],
                                in0=csb[0:64, ds(c * w, w)],
                                in1=recip[:, 0:w], op=MULT)
                        return
                    # read PSUM directly (recip then the normalizing
                    # multiply) -- no staging copy. chunks>1 releases
                    # ctxT columns progressively (tail outproj gating).
                    for c in range(chunks):
                        recip = nrm.tile([64, 512], F32, name="recip")
                        with nc.allow_low_precision(reason="recip"):
                            nc.vector.reciprocal(
                                recip[:, 0:w], jctx[64:128, ds(c * w, w)])
                        nc.vector.tensor_tensor(
                            out=ctxT[pr][hr:hr + 64,
                                         ds(qc * 512 + c * w, w)],
                            in0=jctx[0:64, ds(c * w, w)],
                            in1=recip[:, 0:w], op=MULT)

                def finish_seg(pr, qc):
                    ctx_tiles.pop((pr, qc), None)
                    pt_tiles.pop((pr, qc), None)

                o_state = {}

                def emit_outproj_half(qc, sub, oc, tail=False):
                    """Half an output-projection unit: one 512-col block
                    of a 128-token tile (2 matmuls, 1 shr slot).

                    OP_DMA=psum DMAs the f32 PSUM block straight to DRAM
                    (no evacuation copy at all -- the host sums partials
                    in f32 anyway); otherwise evacuate to SBUF bf16 on
                    DVE/ACT and DMA with the second half."""
                    if "o" in ABL:
                        return
                    ti = qc * 4 + sub
                    o_ps = shr.tile([128, 512], F32, name="shr_ps",
                                    tag="shr")
                    for hc in range(2):
                        lw = (pt_const[:, 0:128] if "c" in ABL
                              else ctxT[hc][:, ts(ti, 128)])
                        nc.tensor.matmul(
                            o_ps, lhsT=lw,
                            rhs=wo_sb[:, hc, ts(oc, 512)],
                            start=(hc == 0), stop=(hc == 1))
                    if op_dma == "psum":
                        if "d" not in ABL:
                            nc.sync.dma_start(
                                out[ti * 128:(ti + 1) * 128,
                                    ds(oc * 512, 512)], o_ps)
                        return
                    if oc == 0:
                        o_state[ti] = outp.tile([128, HID], BF16,
                                                name="o_sb")
                    o_sb = o_state.pop(ti) if oc == 1 else o_state[ti]
                    op_evac = os.environ.get("OP_EVAC", "dve")
                    opch = int(os.environ.get("OPCH", "1"))
                    if (op_evac == "act" or (tail and oc == 1)
                            or (op_evac == "alt" and oc == 1)):
                        nc.scalar.activation(o_sb[:, ts(oc, 512)],
                                             o_ps, COPY)
                    else:
                        cw = 512 // opch
                        for cc in range(opch):
                            nc.vector.tensor_copy(
                                o_sb[:, ds(oc * 512 + cc * cw, cw)],
                                o_ps[:, ds(cc * cw, cw)])
                    if oc == 1 and "d" not in ABL:
                        nc.sync.dma_start(out[ts(ti, 128)], o_sb)

                # ---------- exp-paced quantum schedule ----------
                def seg(sn):
                    return (sn % 2, sn // 2)    # (pr, qc)

                # PV sub-chain split points (3 sub-units per column)
                c1 = (NKT + 2) // 3
                c2 = c1 + (NKT + 1) // 3
                pv_cuts = [(0, c1), (c1, c2), (c2, NKT)]

                QUANT = int(os.environ.get("QUANT", "1400"))
                nchunk = int(os.environ.get("NCHUNK", "2"))
                pvgran = int(os.environ.get("PVGRAN", "3"))

                def build_queue(par_next):
                    """Filler units (thunk, est_ns, n_shr_allocs)
                    projecting K/Q/V for the NEXT rep; ordering keeps
                    deadlines loose."""
                    jobs = kq_jobs()
                    kj = [j for j in jobs if j[0] == "k"]
                    qj = [j for j in jobs if j[0] == "q"]
                    units = []
                    for j in kj:
                        units.append((lambda jj=j:
                                      emit_kq_job(par_next, 1, jj),
                                      1720, 1))
                    units.append((lambda jj=qj[0]:
                                  emit_kq_job(par_next, 1, jj), 1720, 1))
                    for kt in range(NKT):
                        units.append(
                            (lambda t=kt: emit_v(par_next, t), 860, 1))
                    for j in qj[1:]:
                        units.append((lambda jj=j:
                                      emit_kq_job(par_next, 1, jj),
                                      1720, 1))
                    for j in kj + qj:
                        units.append((lambda jj=j:
                                      emit_kq_job(par_next, 0, jj),
                                      1720, 1))
                    return units

                for rep in range(reps):
                    par = rep % 2
                    if rep == 0:
                        # prologue: this rep's own projections (psS is
                        # free before the S stream starts), then K1 +
                        # Q1-qc0 on shr; V rides the first fillers
                        jobs = kq_jobs()
                        for j in jobs:
                            emit_kq_job(par, 0, j, pre=True)
                        kj = [j for j in jobs if j[0] == "k"]
                        qj = [j for j in jobs if j[0] == "q"]
                        for j in kj + qj[:1]:
                            emit_kq_job(par, 1, j)
                        queue = [(lambda t=kt: emit_v(par, t), 860, 1)
                                 for kt in range(NKT)]
                        queue += [(lambda jj=j: emit_kq_job(par, 1, jj),
                                   1720, 1) for j in qj[1:]]
                    else:
                        queue = []
                    if rep + 1 < reps:
                        queue += build_queue((rep + 1) % 2)
                    qi = 0
                    credit = 0.0

                    for k in range(8):
                        pinned = {}
                        if k >= 1:
                            pr1, qc1 = seg(k - 1)
                            if pvgran == 1:
                                for i2 in range(NKT):
                                    for cl in range(2):
                                        pinned.setdefault(i2, []).append(
                                            (lambda a=i2, c=cl:
                                             emit_pv_sub(par, pr1, qc1,
                                                         c, a, a + 1),
                                             220, 0))
                                pinned.setdefault(NKT - 1, []).append(
                                    (lambda p=pr1, q=qc1:
                                     (emit_norm_col(p, q, 0, nchunk),
                                      emit_norm_col(p, q, 1, nchunk),
                                      finish_seg(p, q)), 0, 0))
                            else:
                                for n, (lo, hi) in enumerate(pv_cuts):
                                    pinned.setdefault(n, []).append(
                                        (lambda a=lo, b=hi:
                                         emit_pv_sub(par, pr1, qc1, 0, a, b),
                                         220 * (hi - lo), 0))
                                    pinned.setdefault(3 + n, []).append(
                                        (lambda a=lo, b=hi:
                                         emit_pv_sub(par, pr1, qc1, 1, a, b),
                                         220 * (hi - lo), 0))
                                pinned.setdefault(2, []).append(
                                    (lambda: emit_norm_col(pr1, qc1, 0,
                                                           nchunk),
                                     0, 0))
                                pinned.setdefault(5, []).append(
                                    (lambda p=pr1, q=qc1:
                                     (emit_norm_col(p, q, 1, nchunk),
                                      finish_seg(p, q)), 0, 0))
                        # outproj(qc) halves spread over slots 2qc+3 and
                        # 2qc+4 at spaced positions: one shr slot each,
                        # alternating evac engines (qc2 packs into slot
                        # 7; qc3 is the tail)
                        if k >= 3:
                            oqc = (k - 3) // 2
                            if k == 7:
                                places = [(2, p // 2, p % 2, pos)
                                          for p, pos in zip(
                                              range(4), (1, 3, 5, 7))]
                            else:
                                base = 0 if k % 2 == 1 else 2
                                places = [(oqc, base + p // 2, p % 2, pos)
                                          for p, pos in zip(
                                              range(4), (1, 3, 5, 7))]
                            for q, sub, oc, pos in places:
                                pinned.setdefault(
                                    min(pos, NKT - 1), []).append(
                                    (lambda qq=q, s=sub, o=oc:
                                     emit_outproj_half(qq, s, o),
                                     440, 1))
                        for i in range(NKT):
                            emit_s(par, *seg(k), i)
                            credit += QUANT - 230
                            allocs = 0
                            for fn, cost, na in pinned.get(i, ()):
                                fn()
                                credit -= cost
                                allocs += na
                            # at most one shr-slot allocation per exp
                            # quantum: the 2-deep rotation then never
                            # waits on a just-queued evacuation
                            while (credit > 0 and qi < len(queue)
                                   and allocs + queue[qi][2] <= 1):
                                fn, cost, na = queue[qi]
                                qi += 1
                                fn()
                                credit -= cost
                                allocs += na

                    # dense tail: last segment's PV/norm + outproj(3),
                    # evacs split DVE/ACT; then drain leftover fillers
                    pr1, qc1 = seg(7)
                    for lo, hi in pv_cuts:
                        emit_pv_sub(par, pr1, qc1, 0, lo, hi)
                    emit_norm_col(pr1, qc1, 0, chunks=4)
                    for lo, hi in pv_cuts:
                        emit_pv_sub(par, pr1, qc1, 1, lo, hi)
                    emit_norm_col(pr1, qc1, 1, chunks=4)
                    finish_seg(pr1, qc1)
                    for qq, sub in ((2, 2), (2, 3), (3, 0), (3, 1),
                                    (3, 2), (3, 3)):
                        for oc in range(2):
                            # rep tail: ACT is idle after the last exp --
                            # split the evacuation drain across engines
                            emit_outproj_half(qq, sub, oc, tail=True)
                    while qi < len(queue):
                        fn, cost, na = queue[qi]
                        qi += 1
                        fn()

    nc.compile()
    return nc


_NC = None


def shard_inputs(x, mask, qkv_w, qkv_b, out_w):
    bf = ml_dtypes.bfloat16
    in_maps = []
    for c in range(N_CORES):
        b, g = c // 4, c % 4
        hs = slice(DH * g, DH * (g + 1))
        xTc = np.ascontiguousarray(x[b].T).astype(bf)
        idx = np.where(mask[b] != 0)[0]
        assert len(idx) <= KC, f"unmasked {len(idx)} > KC={KC}"
        xk = np.zeros((KC, HID), np.float32)
        xk[:len(idx)] = x[b][idx]
        xTkc = np.ascontiguousarray(xk.T).astype(bf)
        ones = np.zeros(KC, np.float32)
        ones[:len(idx)] = 1.0
        onekt = ones.reshape(NKT, 128).T           # [key, kt]
        vone64 = np.ascontiguousarray(
            np.repeat(onekt[:, :, None], 64, axis=2)).astype(bf)
        qb = qkv_b[hs]
        kb = qkv_b[1024 + DH * g:1024 + DH * (g + 1)]
        vb = qkv_b[2048 + DH * g:2048 + DH * (g + 1)]
        qkvbT = np.stack([qb[0:128], qb[128:256],
                          kb[0:128], kb[128:256]], axis=1).astype(np.float32)
        vbias = np.ascontiguousarray(
            onekt[:, :, None] * vb[None, None, :]).astype(bf)
        wq = np.ascontiguousarray(qkv_w[hs, :].T).astype(bf)
        wk = np.ascontiguousarray(qkv_w[1024 + DH * g:1024 + DH * (g + 1), :].T
                                  ).astype(bf)
        wv = np.ascontiguousarray(qkv_w[2048 + DH * g:2048 + DH * (g + 1), :].T
                                  ).astype(bf)
        wo = np.ascontiguousarray(out_w[:, hs].T).astype(bf)
        in_maps.append({"xT": xTc, "xTk": xTkc, "wqT": wq, "wkT": wk,
                        "wvT": wv, "woT": wo, "qkvbT": qkvbT,
                        "vbias": vbias, "vone64": vone64})
    return in_maps


def run(in_maps, **kwargs):
    global _NC
    if _NC is None:
        _NC = build_program()
    return bass_utils.run_bass_kernel_spmd(
        _NC, in_maps, core_ids=list(range(N_CORES)), **kwargs)


def kernel(x, mask, qkv_w, qkv_b, out_w, out_b):
    global KC, NKT, _NC
    x = np.asarray(x)
    mask = np.asarray(mask)
    need = int(np.max(np.sum(mask != 0, axis=1)))
    kc = max(128, ((need + 127) // 128) * 128)
    if kc != KC:
        KC, NKT = kc, kc // 128
        _NC = None
    qkv_w = np.asarray(qkv_w)
    qkv_b = np.asarray(qkv_b)
    out_w = np.asarray(out_w)
    out_b = np.asarray(out_b)
    in_maps = shard_inputs(x, mask, qkv_w, qkv_b, out_w)
    res = run(in_maps)
    parts = [r["out"] for r in res.results]
    full = np.empty((2, S, HID), np.float32)
    for b in range(2):
        acc = parts[4 * b].astype(np.float32)
        for g in range(1, 4):
            acc = acc + parts[4 * b + g].astype(np.float32)
        full[b] = acc + out_b[None, :]
    return full


# revision 35
# speedup vs baseline: 1.0097x; 1.0097x over previous
"""Multi-head attention forward, sharded over 8 NeuronCores.

Sharding: batch (2) x head-group (4 groups of 4 heads) = 8 cores.
Each core computes, for its batch b and its 4 heads:
  Q^T/K^T = W^T-slices @ x^T (+bias via per-partition tensor_scalar add),
  V token-major over compacted keys (+bias via a pad-masked bias input
  added during evac -- pad key columns of xTk are zeroed host-side so
  pad keys have V=0 and denominator-weight 0 and drop out of softmax),
  S^T[k,q] = K^T.T@Q^T per k-tile (scores transposed so exp output feeds
  P.V directly), P^T = exp(scale*S^T) with no mask bias,
  ctx_aug^T = [V|1x64]^T.T @ P^T -- the ones block is replicated 64 wide
  so PSUM partitions 64:128 all hold the softmax denominator, i.e. the
  denominator arrives pre-broadcast and normalization is reciprocal +
  multiply reading PSUM directly on DVE,
  out_partial = ctx^T.T @ W_o^T-slice  ->  [2048, 1024] bf16.
Host sums the 4 partials per batch (fp32) and adds out_b.

Schedule: the ACT exp stream (72 x [128,1024] tiles, ~1.04us each) is
the pacing resource; the PE must never micro-idle or the HW HAM
throttle drops the array to half throughput.  Emission is therefore
exp-paced: each S^T tile-pair (the two 64-contraction head matmuls run
CONCURRENTLY on HW via disjoint tile_position row groups, ~220ns) is
topped up with ~1us of dependency-free PE filler.  K/Q/V projections
are double-buffered by rep parity so the NEXT rep's projections float
freely as filler anywhere in the current rep; PV runs as 3-matmul
sub-chains pinned to the slot after its segment's exp; outproj(qc)
lands in slot 2qc+3 (qc-major segment order => both prs' norms for qc
are done by then); the last segment's PV/norm/outproj form a short
dense tail whose evacuations split across DVE and ACT.
"""

import os
import sys

if "/opt/trn_rl_repo" not in sys.path:
    sys.path.insert(0, "/opt/trn_rl_repo")

import numpy as np
import ml_dtypes

import concourse.bass as bass
import concourse.mybir as mybir
from concourse import bacc
from concourse.bass import ts, ds
from concourse.tile import TileContext
from concourse import bass_utils

BF16 = mybir.dt.bfloat16
F32 = mybir.dt.float32
F32R = mybir.dt.float32r
EXP = mybir.ActivationFunctionType.Exp
COPY = mybir.ActivationFunctionType.Copy
MULT = mybir.AluOpType.mult

N_CORES = 8
S = 2048          # sequence length (one batch per core)
HID = 1024
DH = 256          # head dims per core (4 heads x 64)
D = 64
KC = 1280         # compacted+padded key length; runtime-adjusted in kernel()
NKT = KC // 128


def build_program(reps=1):
    ABL = os.environ.get("ABLATE", "")
    nc = bacc.Bacc("TRN2", target_bir_lowering=False, debug=False,
                   num_devices=N_CORES)
    xT = nc.dram_tensor("xT", [HID, S], BF16, kind="ExternalInput").ap()
    xTk = nc.dram_tensor("xTk", [HID, KC], BF16, kind="ExternalInput").ap()
    wqT = nc.dram_tensor("wqT", [HID, DH], BF16, kind="ExternalInput").ap()
    wkT = nc.dram_tensor("wkT", [HID, DH], BF16, kind="ExternalInput").ap()
    wvT = nc.dram_tensor("wvT", [HID, DH], BF16, kind="ExternalInput").ap()
    woT = nc.dram_tensor("woT", [DH, HID], BF16, kind="ExternalInput").ap()
    # per-partition Q/K bias columns: [q pr0, q pr1, k pr0, k pr1]
    qkvbT = nc.dram_tensor("qkvbT", [128, 4], F32,
                           kind="ExternalInput").ap()
    # V bias replicated per key, zeroed on pad keys: [key, kt, 4*64]
    vbias = nc.dram_tensor("vbias", [128, NKT, DH], BF16,
                           kind="ExternalInput").ap()
    # denominator ones block: 1.0 for real keys, 0.0 for pads, x64 wide
    vone64 = nc.dram_tensor("vone64", [128, NKT, 64], BF16,
                            kind="ExternalInput").ap()
    op_dma = os.environ.get("OP_DMA", "sbuf")
    out_dt = F32 if op_dma == "psum" else BF16
    out = nc.dram_tensor("out", [S, HID], out_dt,
                         kind="ExternalOutput").ap()

    with TileContext(nc) as tc:
        with tc.tile_pool(name="const", bufs=1) as cp:
            wq_sb = cp.tile([128, 8, DH], BF16, name="wq_sb")
            wk_sb = cp.tile([128, 8, DH], BF16, name="wk_sb")
            wv_sb = cp.tile([128, 8, DH], BF16, name="wv_sb")
            nc.sync.dma_start(wq_sb, wqT.rearrange("(c p) m -> p c m", p=128))
            nc.sync.dma_start(wk_sb, wkT.rearrange("(c p) m -> p c m", p=128))
            nc.sync.dma_start(wv_sb, wvT.rearrange("(c p) m -> p c m", p=128))
            wo_sb = cp.tile([128, 2, HID], BF16, name="wo_sb")
            nc.sync.dma_start(wo_sb, woT.rearrange("(c p) o -> p c o", p=128))
            qkvbT_sb = cp.tile([128, 4], F32, name="qkvbT_sb")
            nc.sync.dma_start(qkvbT_sb, qkvbT)
            vbias_sb = cp.tile([128, NKT, DH], BF16, name="vbias_sb")
            nc.sync.dma_start(vbias_sb, vbias)

            xt_sb = cp.tile([128, 8, S], BF16, name="xt_sb")
            xt_view = xT.rearrange("(c p) t -> c p t", p=128)
            xtk_sb = cp.tile([128, 8, KC], BF16, name="xtk_sb")
            xtk_view = xTk.rearrange("(c p) t -> c p t", p=128)
            for c in range(8):
                nc.sync.dma_start(xt_sb[:, c, :], xt_view[c])
                nc.sync.dma_start(xtk_sb[:, c, :], xtk_view[c])

            # K/Q/V double-buffered by rep parity: the NEXT rep's
            # projections are emitted as filler anywhere in the current
            # rep with no WAR coupling to this rep's attention reads.
            q_sb = [[cp.tile([128, S], BF16, name=f"q_sb{p}{j}")
                     for j in range(2)] for p in range(2)]
            k_sb = [[cp.tile([128, KC], BF16, name=f"k_sb{p}{j}")
                     for j in range(2)] for p in range(2)]
            # [key-in-tile, kt, head, 64 V dims + 64 denominator ones]
            v_sb = [cp.tile([128, NKT, 4, 128], BF16, name=f"v_sb{p}")
                    for p in range(2)]
            for p in range(2):
                for h in range(4):
                    nc.sync.dma_start(v_sb[p][:, :, h, 64:128], vone64)
            ctxT = [cp.tile([128, S], BF16, name=f"ctxT{j}") for j in range(2)]
            if ABL:                      # keep read-before-write legal
                for p in range(2):
                    for j in range(2):
                        nc.vector.memset(q_sb[p][j], 0.01)
                        nc.vector.memset(k_sb[p][j], 0.01)
                    nc.vector.memset(v_sb[p][:, :, :, 0:64], 0.01)
                for j in range(2):
                    nc.vector.memset(ctxT[j], 0.01)
                sexp_const = cp.tile([128, 1024], F32, name="sexp_const")
                nc.vector.memset(sexp_const, 0.01)
                pt_const = cp.tile([128, 1024], BF16, name="pt_const")
                nc.vector.memset(pt_const, 0.01)

            with tc.tile_pool(name="psS", bufs=2, space="PSUM") as psS, \
                 tc.tile_pool(name="psX", bufs=2, space="PSUM") as psX, \
                 tc.tile_pool(name="shr", bufs=2, space="PSUM") as shr, \
                 tc.tile_pool(name="ptp",
                              bufs=int(os.environ.get("PT_BUFS", "20"))) \
                     as ptp, \
                 tc.tile_pool(name="nrm", bufs=7) as nrm, \
                 tc.tile_pool(name="outp", bufs=3) as outp:

                # PE warmup: ~6us of dummy matmuls that depend only on
                # a local memset, so the array ramps to full clock (HAM
                # K=8/8) while the input DMAs are still landing instead
                # of entering the real stream cold.
                nwarm = int(os.environ.get("WARM", "30"))
                if nwarm:
                    warm = cp.tile([128, 512], BF16, name="warm")
                    nc.vector.memset(warm, 0.5)
                    wps = psS.tile([128, 1024], F32, name="s_ps")
                    for i in range(nwarm):
                        nc.tensor.matmul(wps[:, 0:512],
                                         lhsT=warm[:, 0:128], rhs=warm,
                                         start=(i == 0),
                                         stop=(i == nwarm - 1))

                # ---------- emission helpers ----------
                def kq_jobs():
                    jobs = []
                    off = 0
                    while off < KC:           # K chunks (compact tokens)
                        w = min(512, KC - off)
                        jobs.append(("k", off, w))
                        off += w
                    for n in range(4):        # Q chunks (all tokens)
                        jobs.append(("q", n * 512, 512))
                    return jobs

                IDENT = mybir.ActivationFunctionType.Identity
                kq_ctr = [0]

                def emit_kq_job(par, pr, job, pre=False):
                    """One K/Q projection chunk (8 matmuls + bias evac).
                    Evacuation alternates DVE tensor_scalar / ACT
                    Identity+bias so consecutive shr slots never wait on
                    the same engine."""
                    if "q" in ABL:
                        return
                    kind, off, w = job
                    if kind == "k":
                        src_sb, w_sb = xtk_sb, wk_sb
                        dst, bcol = k_sb[par][pr], 2 + pr
                    else:
                        src_sb, w_sb = xt_sb, wq_sb
                        dst, bcol = q_sb[par][pr], pr
                    if pre:
                        ps = psS.tile([128, 1024], F32, name="s_ps")
                    else:
                        ps = shr.tile([128, 512], F32,
                                      name="shr_ps", tag="shr")
                    for c in range(8):
                        nc.tensor.matmul(
                            ps[:, 0:w],
                            lhsT=w_sb[:, c, ts(pr, 128)],
                            rhs=src_sb[:, c, ds(off, w)],
                            start=(c == 0), stop=(c == 7))
                    kq_ctr[0] += 1
                    kq_evac = os.environ.get("KQ_EVAC", "dve")
                    kqch = int(os.environ.get("KQCH", "1"))
                    if (kq_evac == "act"
                            or (kq_evac == "alt" and kq_ctr[0] % 2 == 0)):
                        nc.scalar.activation(
                            dst[:, ds(off, w)], ps[:, 0:w], IDENT,
                            bias=qkvbT_sb[:, bcol:bcol + 1])
                    else:
                        cw = w // kqch
                        for cc in range(kqch):
                            nc.vector.tensor_scalar_add(
                                dst[:, ds(off + cc * cw, cw)],
                                ps[:, ds(cc * cw, cw)],
                                qkvbT_sb[:, bcol:bcol + 1])

                def emit_v(par, kt):
                    """V projection for one key tile, all 4 heads."""
                    if "v" in ABL:
                        return
                    ps = shr.tile([128, 512], F32, name="shr_ps", tag="shr")
                    for c in range(8):
                        nc.tensor.matmul(ps[:, 0:DH],
                                         lhsT=xtk_sb[:, c, ts(kt, 128)],
                                         rhs=wv_sb[:, c, :],
                                         start=(c == 0), stop=(c == 7))
                    nc.vector.tensor_tensor(
                        out=v_sb[par][:, kt, :, 0:64],
                        in0=ps[:, 0:DH].rearrange("p (h d) -> p h d", h=4),
                        in1=vbias_sb[:, kt, :].rearrange(
                            "p (h d) -> p h d", h=4),
                        op=mybir.AluOpType.add)

                # ---------- attention helpers ----------
                pt_tiles = {}     # (pr, qc) -> list of pt tiles

                def emit_s(par, pr, qc, kt):
                    if "s" not in ABL:
                        s_ps = psS.tile([128, 1024], F32, name="s_ps")
                        for col in range(2):
                            hr = col * 64
                            nc.tensor.matmul(
                                s_ps[:, ts(col, 512)],
                                lhsT=k_sb[par][pr][hr:hr + 64, ts(kt, 128)],
                                rhs=q_sb[par][pr][hr:hr + 64,
                                                  ds(qc * 512, 512)],
                                start=True, stop=True,
                                tile_position=(hr, 0))
                    else:
                        s_ps = sexp_const
                    if "e" not in ABL:
                        pt = ptp.tile([128, 1024], BF16, name="pt")
                        nc.scalar.activation(pt, s_ps, EXP, scale=0.125)
                    else:
                        pt = pt_const
                    pt_tiles.setdefault((pr, qc), []).append(pt)

                ctx_tiles = {}    # (pr, qc) -> (ctxA, ctxB)

                def emit_pv_sub(par, pr, qc, col, klo, khi):
                    """Part of one head's accumulation chain (kt klo:khi)."""
                    if "p" in ABL:
                        return
                    if col == 0 and klo == 0:
                        ctx_tiles[(pr, qc)] = (
                            psX.tile([128, 512], F32, name="ctx_ps"),
                            psX.tile([128, 512], F32, name="ctx_ps"))
                    jctx = ctx_tiles[(pr, qc)][col]
                    pts = pt_tiles[(pr, qc)]
                    for kt in range(klo, khi):
                        nc.tensor.matmul(
                            jctx, lhsT=v_sb[par][:, kt, 2 * pr + col, :],
                            rhs=pts[kt][:, ts(col, 512)],
                            start=(kt == 0), stop=(kt == NKT - 1))

                nstage = int(os.environ.get("NSTAGE", "0"))

                def emit_norm_col(pr, qc, col, chunks=1):
                    if "n" in ABL or "p" in ABL:
                        return
                    jctx = ctx_tiles[(pr, qc)][col]
                    # partitions 64:128 already hold the denominator.
                    hr = col * 64
                    w = 512 // chunks
                    if nstage:
                        # stage through ACT to bf16 SBUF: the DVE recip +
                        # multiply then run in 2-byte 2x/4x perf modes
                        # (~4x cheaper on the coupling engine); bf16
                        # denominator costs ~0.4% rel err (budget 2e-2)
                        csb = nrm.tile([128, 512], BF16, name="csb")
                        nc.scalar.activation(csb, jctx, COPY)
                        for c in range(chunks):
                            recip = nrm.tile([64, 512], BF16, name="recip")
                            with nc.allow_low_precision(reason="recip"):
                                nc.vector.reciprocal(
                                    recip[:, 0:w],
                                    csb[64:128, ds(c * w, w)])
                            nc.vector.tensor_tensor(
                                out=ctxT[pr][hr:hr + 64,
                                             ds(qc * 512 + c * w, w)],
                                in0=csb[0:64, ds(c * w, w)],
                                in1=recip[:, 0:w], op=MULT)
                        return
                    # read PSUM directly (recip then the normalizing
                    # multiply) -- no staging copy. chunks>1 releases
                    # ctxT columns progressively (tail outproj gating).
                    for c in range(chunks):
                        recip = nrm.tile([64, 512], F32, name="recip")
                        with nc.allow_low_precision(reason="recip"):
                            nc.vector.reciprocal(
                                recip[:, 0:w], jctx[64:128, ds(c * w, w)])
                        nc.vector.tensor_tensor(
                            out=ctxT[pr][hr:hr + 64,
                                         ds(qc * 512 + c * w, w)],
                            in0=jctx[0:64, ds(c * w, w)],
                            in1=recip[:, 0:w], op=MULT)

                def finish_seg(pr, qc):
                    ctx_tiles.pop((pr, qc), None)
                    pt_tiles.pop((pr, qc), None)

                o_state = {}

                def emit_outproj_half(qc, sub, oc, tail=False):
                    """Half an output-projection unit: one 512-col block
                    of a 128-token tile (2 matmuls, 1 shr slot).

                    OP_DMA=psum DMAs the f32 PSUM block straight to DRAM
                    (no evacuation copy at all -- the host sums partials
                    in f32 anyway); otherwise evacuate to SBUF bf16 on
                    DVE/ACT and DMA with the second half."""
                    if "o" in ABL:
                        return
                    ti = qc * 4 + sub
                    o_ps = shr.tile([128, 512], F32, name="shr_ps",
                                    tag="shr")
                    for hc in range(2):
                        lw = (pt_const[:, 0:128] if "c" in ABL
                              else ctxT[hc][:, ts(ti, 128)])
                        nc.tensor.matmul(
                            o_ps, lhsT=lw,
                            rhs=wo_sb[:, hc, ts(oc, 512)],
                            start=(hc == 0), stop=(hc == 1))
                    if op_dma == "psum":
                        if "d" not in ABL:
                            nc.sync.dma_start(
                                out[ti * 128:(ti + 1) * 128,
                                    ds(oc * 512, 512)], o_ps)
                        return
                    if oc == 0:
                        o_state[ti] = outp.tile([128, HID], BF16,
                                                name="o_sb")
                    o_sb = o_state.pop(ti) if oc == 1 else o_state[ti]
                    op_evac = os.environ.get("OP_EVAC", "dve")
                    opch = int(os.environ.get("OPCH", "1"))
                    if (op_evac == "act" or (tail and oc == 1)
                            or (op_evac == "alt" and oc == 1)):
                        nc.scalar.activation(o_sb[:, ts(oc, 512)],
                                             o_ps, COPY)
                    else:
                        cw = 512 // opch
                        for cc in range(opch):
                            nc.vector.tensor_copy(
                                o_sb[:, ds(oc * 512 + cc * cw, cw)],
                                o_ps[:, ds(cc * cw, cw)])
                    if oc == 1 and "d" not in ABL:
                        nc.sync.dma_start(out[ts(ti, 128)], o_sb)

                # ---------- exp-paced quantum schedule ----------
                def seg(sn):
                    return (sn % 2, sn // 2)    # (pr, qc)

                # PV sub-chain split points (3 sub-units per column)
                c1 = (NKT + 2) // 3
                c2 = c1 + (NKT + 1) // 3
                pv_cuts = [(0, c1), (c1, c2), (c2, NKT)]

                QUANT = int(os.environ.get("QUANT", "1400"))
                nchunk = int(os.environ.get("NCHUNK", "2"))
                pvgran = int(os.environ.get("PVGRAN", "3"))

                def build_queue(par_next):
                    """Filler units (thunk, est_ns, n_shr_allocs)
                    projecting K/Q/V for the NEXT rep; ordering keeps
                    deadlines loose."""
                    jobs = kq_jobs()
                    kj = [j for j in jobs if j[0] == "k"]
                    qj = [j for j in jobs if j[0] == "q"]
                    units = []
                    for j in kj:
                        units.append((lambda jj=j:
                                      emit_kq_job(par_next, 1, jj),
                                      1720, 1))
                    units.append((lambda jj=qj[0]:
                                  emit_kq_job(par_next, 1, jj), 1720, 1))
                    for kt in range(NKT):
                        units.append(
                            (lambda t=kt: emit_v(par_next, t), 860, 1))
                    for j in qj[1:]:
                        units.append((lambda jj=j:
                                      emit_kq_job(par_next, 1, jj),
                                      1720, 1))
                    for j in kj + qj:
                        units.append((lambda jj=j:
                                      emit_kq_job(par_next, 0, jj),
                                      1720, 1))
                    return units

                for rep in range(reps):
                    par = rep % 2
                    if rep == 0:
                        # prologue: this rep's own projections (psS is
                        # free before the S stream starts), then K1 +
                        # Q1-qc0 on shr; V rides the first fillers
                        jobs = kq_jobs()
                        for j in jobs:
                            emit_kq_job(par, 0, j, pre=True)
                        kj = [j for j in jobs if j[0] == "k"]
                        qj = [j for j in jobs if j[0] == "q"]
                        for j in kj + qj[:1]:
                            emit_kq_job(par, 1, j)
                        queue = [(lambda t=kt: emit_v(par, t), 860, 1)
                                 for kt in range(NKT)]
                        queue += [(lambda jj=j: emit_kq_job(par, 1, jj),
                                   1720, 1) for j in qj[1:]]
                    else:
                        queue = []
                    if rep + 1 < reps:
                        queue += build_queue((rep + 1) % 2)
                    qi = 0
                    credit = 0.0

                    for k in range(8):
                        pinned = {}
                        if k >= 1:
                            pr1, qc1 = seg(k - 1)
                            if pvgran == 1:
                                for i2 in range(NKT):
                                    for cl in range(2):
                                        pinned.setdefault(i2, []).append(
                                            (lambda a=i2, c=cl:
                                             emit_pv_sub(par, pr1, qc1,
                                                         c, a, a + 1),
                                             220, 0))
                                pinned.setdefault(NKT - 1, []).append(
                                    (lambda p=pr1, q=qc1:
                                     (emit_norm_col(p, q, 0, nchunk),
                                      emit_norm_col(p, q, 1, nchunk),
                                      finish_seg(p, q)), 0, 0))
                            else:
                                # PVPIN=1 spreads the six PV sub-chains
                                # across the whole slot (avoids stacking
                                # on the outproj positions 1,3,5) so
                                # per-quantum PE load is even
                                if int(os.environ.get("PVPIN", "1")):
                                    posA, posB = (0, 2, 4), (5, 6, 7)
                                    nA, nB = 4, min(7, NKT - 1)
                                else:
                                    posA, posB = (0, 1, 2), (3, 4, 5)
                                    nA, nB = 2, 5
                                for n, (lo, hi) in enumerate(pv_cuts):
                                    pinned.setdefault(posA[n], []).append(
                                        (lambda a=lo, b=hi:
                                         emit_pv_sub(par, pr1, qc1, 0, a, b),
                                         220 * (hi - lo), 0))
                                    pinned.setdefault(posB[n], []).append(
                                        (lambda a=lo, b=hi:
                                         emit_pv_sub(par, pr1, qc1, 1, a, b),
                                         220 * (hi - lo), 0))
                                pinned.setdefault(nA, []).append(
                                    (lambda: emit_norm_col(pr1, qc1, 0,
                                                           nchunk),
                                     0, 0))
                                pinned.setdefault(nB, []).append(
                                    (lambda p=pr1, q=qc1:
                                     (emit_norm_col(p, q, 1, nchunk),
                                      finish_seg(p, q)), 0, 0))
                        # outproj(qc) halves spread over slots 2qc+3 and
                        # 2qc+4 at spaced positions: one shr slot each,
                        # alternating evac engines (qc2 packs into slot
                        # 7; qc3 is the tail)
                        if k >= 3:
                            oqc = (k - 3) // 2
                            if k == 7:
                                places = [(2, p // 2, p % 2, pos)
                                          for p, pos in zip(
                                              range(4), (1, 3, 5, 7))]
                            else:
                                base = 0 if k % 2 == 1 else 2
                                places = [(oqc, base + p // 2, p % 2, pos)
                                          for p, pos in zip(
                                              range(4), (1, 3, 5, 7))]
                            for q, sub, oc, pos in places:
                                pinned.setdefault(
                                    min(pos, NKT - 1), []).append(
                                    (lambda qq=q, s=sub, o=oc:
                                     emit_outproj_half(qq, s, o),
                                     440, 1))
                        for i in range(NKT):
                            emit_s(par, *seg(k), i)
                            credit += QUANT - 230
                            allocs = 0
                            for fn, cost, na in pinned.get(i, ()):
                                fn()
                                credit -= cost
                                allocs += na
                            # at most one shr-slot allocation per exp
                            # quantum: the 2-deep rotation then never
                            # waits on a just-queued evacuation
                            while (credit > 0 and qi < len(queue)
                                   and allocs + queue[qi][2] <= 1):
                                fn, cost, na = queue[qi]
                                qi += 1
                                fn()
                                credit -= cost
                                allocs += na

                    # dense tail: last segment's PV/norm + outproj(3),
                    # evacs split DVE/ACT; then drain leftover fillers
                    pr1, qc1 = seg(7)
                    for lo, hi in pv_cuts:
                        emit_pv_sub(par, pr1, qc1, 0, lo, hi)
                    emit_norm_col(pr1, qc1, 0, chunks=4)
                    for lo, hi in pv_cuts:
                        emit_pv_sub(par, pr1, qc1, 1, lo, hi)
                    emit_norm_col(pr1, qc1, 1, chunks=4)
                    finish_seg(pr1, qc1)
                    for qq, sub in ((2, 2), (2, 3), (3, 0), (3, 1),
                                    (3, 2), (3, 3)):
                        for oc in range(2):
                            # rep tail: ACT is idle after the last exp --
                            # split the evacuation drain across engines
                            emit_outproj_half(qq, sub, oc, tail=True)
                    while qi < len(queue):
                        fn, cost, na = queue[qi]
                        qi += 1
                        fn()

    nc.compile()
    return nc


_NC = None


def shard_inputs(x, mask, qkv_w, qkv_b, out_w):
    bf = ml_dtypes.bfloat16
    in_maps = []
    for c in range(N_CORES):
        b, g = c // 4, c % 4
        hs = slice(DH * g, DH * (g + 1))
        xTc = np.ascontiguousarray(x[b].T).astype(bf)
        idx = np.where(mask[b] != 0)[0]
        assert len(idx) <= KC, f"unmasked {len(idx)} > KC={KC}"
        xk = np.zeros((KC, HID), np.float32)
        xk[:len(idx)] = x[b][idx]
        xTkc = np.ascontiguousarray(xk.T).astype(bf)
        ones = np.zeros(KC, np.float32)
        ones[:len(idx)] = 1.0
        onekt = ones.reshape(NKT, 128).T           # [key, kt]
        vone64 = np.ascontiguousarray(
            np.repeat(onekt[:, :, None], 64, axis=2)).astype(bf)
        qb = qkv_b[hs]
        kb = qkv_b[1024 + DH * g:1024 + DH * (g + 1)]
        vb = qkv_b[2048 + DH * g:2048 + DH * (g + 1)]
        qkvbT = np.stack([qb[0:128], qb[128:256],
                          kb[0:128], kb[128:256]], axis=1).astype(np.float32)
        vbias = np.ascontiguousarray(
            onekt[:, :, None] * vb[None, None, :]).astype(bf)
        wq = np.ascontiguousarray(qkv_w[hs, :].T).astype(bf)
        wk = np.ascontiguousarray(qkv_w[1024 + DH * g:1024 + DH * (g + 1), :].T
                                  ).astype(bf)
        wv = np.ascontiguousarray(qkv_w[2048 + DH * g:2048 + DH * (g + 1), :].T
                                  ).astype(bf)
        wo = np.ascontiguousarray(out_w[:, hs].T).astype(bf)
        in_maps.append({"xT": xTc, "xTk": xTkc, "wqT": wq, "wkT": wk,
                        "wvT": wv, "woT": wo, "qkvbT": qkvbT,
                        "vbias": vbias, "vone64": vone64})
    return in_maps


def run(in_maps, **kwargs):
    global _NC
    if _NC is None:
        _NC = build_program()
    return bass_utils.run_bass_kernel_spmd(
        _NC, in_maps, core_ids=list(range(N_CORES)), **kwargs)


def kernel(x, mask, qkv_w, qkv_b, out_w, out_b):
    global KC, NKT, _NC
    x = np.asarray(x)
    mask = np.asarray(mask)
    need = int(np.max(np.sum(mask != 0, axis=1)))
    kc = max(128, ((need + 127) // 128) * 128)
    if kc != KC:
        KC, NKT = kc, kc // 128
        _NC = None
    qkv_w = np.asarray(qkv_w)
    qkv_b = np.asarray(qkv_b)
    out_w = np.asarray(out_w)
    out_b = np.asarray(out_b)
    in_maps = shard_inputs(x, mask, qkv_w, qkv_b, out_w)
    res = run(in_maps)
    parts = [r["out"] for r in res.results]
    full = np.empty((2, S, HID), np.float32)
    for b in range(2):
        acc = parts[4 * b].astype(np.float32)
        for g in range(1, 4):
            acc = acc + parts[4 * b + g].astype(np.float32)
        full[b] = acc + out_b[None, :]
    return full


# revision 36
# speedup vs baseline: 1.0943x; 1.0837x over previous
"""Multi-head attention forward, sharded over 8 NeuronCores.

Sharding: batch (2) x head-group (4 groups of 4 heads) = 8 cores.
Each core computes, for its batch b and its 4 heads:
  Q^T/K^T = W^T-slices @ x^T (+bias via per-partition tensor_scalar add),
  V token-major over compacted keys (+bias via a pad-masked bias input
  added during evac -- pad key columns of xTk are zeroed host-side so
  pad keys have V=0 and denominator-weight 0 and drop out of softmax),
  S^T[k,q] = K^T.T@Q^T per k-tile (scores transposed so exp output feeds
  P.V directly), P^T = exp(scale*S^T) with no mask bias,
  ctx_aug^T = [V|1x64]^T.T @ P^T -- the ones block is replicated 64 wide
  so PSUM partitions 64:128 all hold the softmax denominator, i.e. the
  denominator arrives pre-broadcast and normalization is reciprocal +
  multiply reading PSUM directly on DVE,
  out_partial = ctx^T.T @ W_o^T-slice  ->  [2048, 1024] bf16.
Host sums the 4 partials per batch (fp32) and adds out_b.

Schedule: the ACT exp stream (72 x [128,1024] tiles, ~1.04us each) is
the pacing resource; the PE must never micro-idle or the HW HAM
throttle drops the array to half throughput.  Emission is therefore
exp-paced: each S^T tile-pair (the two 64-contraction head matmuls run
CONCURRENTLY on HW via disjoint tile_position row groups, ~220ns) is
topped up with ~1us of dependency-free PE filler.  K/Q/V projections
are double-buffered by rep parity so the NEXT rep's projections float
freely as filler anywhere in the current rep; PV runs as 3-matmul
sub-chains pinned to the slot after its segment's exp; outproj(qc)
lands in slot 2qc+3 (qc-major segment order => both prs' norms for qc
are done by then); the last segment's PV/norm/outproj form a short
dense tail whose evacuations split across DVE and ACT.
"""

import os
import sys

if "/opt/trn_rl_repo" not in sys.path:
    sys.path.insert(0, "/opt/trn_rl_repo")

import numpy as np
import ml_dtypes

import concourse.bass as bass
import concourse.mybir as mybir
from concourse import bacc
from concourse.bass import ts, ds
from concourse.tile import TileContext
from concourse import bass_utils

BF16 = mybir.dt.bfloat16
F32 = mybir.dt.float32
F32R = mybir.dt.float32r
EXP = mybir.ActivationFunctionType.Exp
COPY = mybir.ActivationFunctionType.Copy
MULT = mybir.AluOpType.mult

N_CORES = 8
S = 2048          # sequence length (one batch per core)
HID = 1024
DH = 256          # head dims per core (4 heads x 64)
D = 64
KC = 1280         # compacted+padded key length; runtime-adjusted in kernel()
NKT = KC // 128


def build_program(reps=1):
    ABL = os.environ.get("ABLATE", "")
    nc = bacc.Bacc("TRN2", target_bir_lowering=False, debug=False,
                   num_devices=N_CORES)
    xT = nc.dram_tensor("xT", [HID, S], BF16, kind="ExternalInput").ap()
    xTk = nc.dram_tensor("xTk", [HID, KC], BF16, kind="ExternalInput").ap()
    wqT = nc.dram_tensor("wqT", [HID, DH], BF16, kind="ExternalInput").ap()
    wkT = nc.dram_tensor("wkT", [HID, DH], BF16, kind="ExternalInput").ap()
    wvT = nc.dram_tensor("wvT", [HID, DH], BF16, kind="ExternalInput").ap()
    woT = nc.dram_tensor("woT", [DH, HID], BF16, kind="ExternalInput").ap()
    # per-partition Q/K bias columns: [q pr0, q pr1, k pr0, k pr1]
    qkvbT = nc.dram_tensor("qkvbT", [128, 4], F32,
                           kind="ExternalInput").ap()
    # V bias replicated per key, zeroed on pad keys: [key, kt, 4*64]
    vbias = nc.dram_tensor("vbias", [128, NKT, DH], BF16,
                           kind="ExternalInput").ap()
    # denominator ones block: 1.0 for real keys, 0.0 for pads, x64 wide
    vone64 = nc.dram_tensor("vone64", [128, NKT, 64], BF16,
                            kind="ExternalInput").ap()
    op_dma = os.environ.get("OP_DMA", "sbuf")
    out_dt = F32 if op_dma == "psum" else BF16
    out = nc.dram_tensor("out", [S, HID], out_dt,
                         kind="ExternalOutput").ap()

    with TileContext(nc) as tc:
        with tc.tile_pool(name="const", bufs=1) as cp:
            wq_sb = cp.tile([128, 8, DH], BF16, name="wq_sb")
            wk_sb = cp.tile([128, 8, DH], BF16, name="wk_sb")
            wv_sb = cp.tile([128, 8, DH], BF16, name="wv_sb")
            nc.sync.dma_start(wq_sb, wqT.rearrange("(c p) m -> p c m", p=128))
            nc.sync.dma_start(wk_sb, wkT.rearrange("(c p) m -> p c m", p=128))
            nc.sync.dma_start(wv_sb, wvT.rearrange("(c p) m -> p c m", p=128))
            wo_sb = cp.tile([128, 2, HID], BF16, name="wo_sb")
            nc.sync.dma_start(wo_sb, woT.rearrange("(c p) o -> p c o", p=128))
            qkvbT_sb = cp.tile([128, 4], F32, name="qkvbT_sb")
            nc.sync.dma_start(qkvbT_sb, qkvbT)
            vbias_sb = cp.tile([128, NKT, DH], BF16, name="vbias_sb")
            nc.sync.dma_start(vbias_sb, vbias)

            xt_sb = cp.tile([128, 8, S], BF16, name="xt_sb")
            xt_view = xT.rearrange("(c p) t -> c p t", p=128)
            xtk_sb = cp.tile([128, 8, KC], BF16, name="xtk_sb")
            xtk_view = xTk.rearrange("(c p) t -> c p t", p=128)
            for c in range(8):
                nc.sync.dma_start(xt_sb[:, c, :], xt_view[c])
                nc.sync.dma_start(xtk_sb[:, c, :], xtk_view[c])

            # K/Q/V double-buffered by rep parity: the NEXT rep's
            # projections are emitted as filler anywhere in the current
            # rep with no WAR coupling to this rep's attention reads.
            q_sb = [[cp.tile([128, S], BF16, name=f"q_sb{p}{j}")
                     for j in range(2)] for p in range(2)]
            k_sb = [[cp.tile([128, KC], BF16, name=f"k_sb{p}{j}")
                     for j in range(2)] for p in range(2)]
            # [key-in-tile, kt, head, 64 V dims + 64 denominator ones]
            v_sb = [cp.tile([128, NKT, 4, 128], BF16, name=f"v_sb{p}")
                    for p in range(2)]
            for p in range(2):
                for h in range(4):
                    nc.sync.dma_start(v_sb[p][:, :, h, 64:128], vone64)
            ctxT = [cp.tile([128, S], BF16, name=f"ctxT{j}") for j in range(2)]
            if ABL:                      # keep read-before-write legal
                for p in range(2):
                    for j in range(2):
                        nc.vector.memset(q_sb[p][j], 0.01)
                        nc.vector.memset(k_sb[p][j], 0.01)
                    nc.vector.memset(v_sb[p][:, :, :, 0:64], 0.01)
                for j in range(2):
                    nc.vector.memset(ctxT[j], 0.01)
                sexp_const = cp.tile([128, 1024], F32, name="sexp_const")
                nc.vector.memset(sexp_const, 0.01)
                pt_const = cp.tile([128, 1024], BF16, name="pt_const")
                nc.vector.memset(pt_const, 0.01)

            with tc.tile_pool(name="psS", bufs=2, space="PSUM") as psS, \
                 tc.tile_pool(name="psX", bufs=2, space="PSUM") as psX, \
                 tc.tile_pool(name="shr", bufs=2, space="PSUM") as shr, \
                 tc.tile_pool(name="ptp",
                              bufs=int(os.environ.get("PT_BUFS", "20"))) \
                     as ptp, \
                 tc.tile_pool(name="nrm", bufs=7) as nrm, \
                 tc.tile_pool(name="outp", bufs=3) as outp:

                # PE warmup: ~6us of dummy matmuls that depend only on
                # a local memset, so the array ramps to full clock (HAM
                # K=8/8) while the input DMAs are still landing instead
                # of entering the real stream cold.
                nwarm = int(os.environ.get("WARM", "30"))
                if nwarm:
                    warm = cp.tile([128, 512], BF16, name="warm")
                    nc.vector.memset(warm, 0.5)
                    wps = psS.tile([128, 1024], F32, name="s_ps")
                    for i in range(nwarm):
                        nc.tensor.matmul(wps[:, 0:512],
                                         lhsT=warm[:, 0:128], rhs=warm,
                                         start=(i == 0),
                                         stop=(i == nwarm - 1))

                # ---------- emission helpers ----------
                def kq_jobs():
                    jobs = []
                    off = 0
                    while off < KC:           # K chunks (compact tokens)
                        w = min(512, KC - off)
                        jobs.append(("k", off, w))
                        off += w
                    for n in range(4):        # Q chunks (all tokens)
                        jobs.append(("q", n * 512, 512))
                    return jobs

                IDENT = mybir.ActivationFunctionType.Identity
                kq_ctr = [0]

                def emit_kq_job(par, pr, job, pre=False):
                    """One K/Q projection chunk (8 matmuls + bias evac).
                    Evacuation alternates DVE tensor_scalar / ACT
                    Identity+bias so consecutive shr slots never wait on
                    the same engine."""
                    if "q" in ABL:
                        return
                    kind, off, w = job
                    if kind == "k":
                        src_sb, w_sb = xtk_sb, wk_sb
                        dst, bcol = k_sb[par][pr], 2 + pr
                    else:
                        src_sb, w_sb = xt_sb, wq_sb
                        dst, bcol = q_sb[par][pr], pr
                    if pre:
                        ps = psS.tile([128, 1024], F32, name="s_ps")
                    else:
                        ps = shr.tile([128, 512], F32,
                                      name="shr_ps", tag="shr")
                    for c in range(8):
                        nc.tensor.matmul(
                            ps[:, 0:w],
                            lhsT=w_sb[:, c, ts(pr, 128)],
                            rhs=src_sb[:, c, ds(off, w)],
                            start=(c == 0), stop=(c == 7))
                    kq_ctr[0] += 1
                    kq_evac = os.environ.get("KQ_EVAC", "dve")
                    kqch = int(os.environ.get("KQCH", "1"))
                    if (kq_evac == "act"
                            or (kq_evac == "alt" and kq_ctr[0] % 2 == 0)):
                        nc.scalar.activation(
                            dst[:, ds(off, w)], ps[:, 0:w], IDENT,
                            bias=qkvbT_sb[:, bcol:bcol + 1])
                    else:
                        cw = w // kqch
                        for cc in range(kqch):
                            nc.vector.tensor_scalar_add(
                                dst[:, ds(off + cc * cw, cw)],
                                ps[:, ds(cc * cw, cw)],
                                qkvbT_sb[:, bcol:bcol + 1])

                def emit_v(par, kt):
                    """V projection for one key tile, all 4 heads."""
                    if "v" in ABL:
                        return
                    ps = shr.tile([128, 512], F32, name="shr_ps", tag="shr")
                    for c in range(8):
                        nc.tensor.matmul(ps[:, 0:DH],
                                         lhsT=xtk_sb[:, c, ts(kt, 128)],
                                         rhs=wv_sb[:, c, :],
                                         start=(c == 0), stop=(c == 7))
                    nc.vector.tensor_tensor(
                        out=v_sb[par][:, kt, :, 0:64],
                        in0=ps[:, 0:DH].rearrange("p (h d) -> p h d", h=4),
                        in1=vbias_sb[:, kt, :].rearrange(
                            "p (h d) -> p h d", h=4),
                        op=mybir.AluOpType.add)

                # ---------- attention helpers ----------
                pt_tiles = {}     # (pr, qc) -> list of pt tiles

                def emit_s(par, pr, qc, kt):
                    if "s" not in ABL:
                        s_ps = psS.tile([128, 1024], F32, name="s_ps")
                        for col in range(2):
                            hr = col * 64
                            nc.tensor.matmul(
                                s_ps[:, ts(col, 512)],
                                lhsT=k_sb[par][pr][hr:hr + 64, ts(kt, 128)],
                                rhs=q_sb[par][pr][hr:hr + 64,
                                                  ds(qc * 512, 512)],
                                start=True, stop=True,
                                tile_position=(hr, 0))
                    else:
                        s_ps = sexp_const
                    if "e" not in ABL:
                        pt = ptp.tile([128, 1024], BF16, name="pt")
                        nc.scalar.activation(pt, s_ps, EXP, scale=0.125)
                    else:
                        pt = pt_const
                    pt_tiles.setdefault((pr, qc), []).append(pt)

                ctx_tiles = {}    # (pr, qc) -> (ctxA, ctxB)

                def emit_pv_sub(par, pr, qc, col, klo, khi):
                    """Part of one head's accumulation chain (kt klo:khi)."""
                    if "p" in ABL:
                        return
                    if col == 0 and klo == 0:
                        ctx_tiles[(pr, qc)] = (
                            psX.tile([128, 512], F32, name="ctx_ps"),
                            psX.tile([128, 512], F32, name="ctx_ps"))
                    jctx = ctx_tiles[(pr, qc)][col]
                    pts = pt_tiles[(pr, qc)]
                    for kt in range(klo, khi):
                        nc.tensor.matmul(
                            jctx, lhsT=v_sb[par][:, kt, 2 * pr + col, :],
                            rhs=pts[kt][:, ts(col, 512)],
                            start=(kt == 0), stop=(kt == NKT - 1))

                nstage = int(os.environ.get("NSTAGE", "0"))

                def emit_norm_col(pr, qc, col, chunks=1):
                    if "n" in ABL or "p" in ABL:
                        return
                    jctx = ctx_tiles[(pr, qc)][col]
                    # partitions 64:128 already hold the denominator.
                    hr = col * 64
                    w = 512 // chunks
                    if nstage:
                        # stage through ACT to bf16 SBUF: the DVE recip +
                        # multiply then run in 2-byte 2x/4x perf modes
                        # (~4x cheaper on the coupling engine); bf16
                        # denominator costs ~0.4% rel err (budget 2e-2)
                        csb = nrm.tile([128, 512], BF16, name="csb")
                        nc.scalar.activation(csb, jctx, COPY)
                        for c in range(chunks):
                            recip = nrm.tile([64, 512], BF16, name="recip")
                            with nc.allow_low_precision(reason="recip"):
                                nc.vector.reciprocal(
                                    recip[:, 0:w],
                                    csb[64:128, ds(c * w, w)])
                            nc.vector.tensor_tensor(
                                out=ctxT[pr][hr:hr + 64,
                                             ds(qc * 512 + c * w, w)],
                                in0=csb[0:64, ds(c * w, w)],
                                in1=recip[:, 0:w], op=MULT)
                        return
                    # read PSUM directly (recip then the normalizing
                    # multiply) -- no staging copy. chunks>1 releases
                    # ctxT columns progressively (tail outproj gating).
                    for c in range(chunks):
                        recip = nrm.tile([64, 512], F32, name="recip")
                        with nc.allow_low_precision(reason="recip"):
                            nc.vector.reciprocal(
                                recip[:, 0:w], jctx[64:128, ds(c * w, w)])
                        nc.vector.tensor_tensor(
                            out=ctxT[pr][hr:hr + 64,
                                         ds(qc * 512 + c * w, w)],
                            in0=jctx[0:64, ds(c * w, w)],
                            in1=recip[:, 0:w], op=MULT)

                def finish_seg(pr, qc):
                    ctx_tiles.pop((pr, qc), None)
                    pt_tiles.pop((pr, qc), None)

                o_state = {}

                def emit_outproj_half(qc, sub, oc, tail=False):
                    """Half an output-projection unit: one 512-col block
                    of a 128-token tile (2 matmuls, 1 shr slot).

                    OP_DMA=psum DMAs the f32 PSUM block straight to DRAM
                    (no evacuation copy at all -- the host sums partials
                    in f32 anyway); otherwise evacuate to SBUF bf16 on
                    DVE/ACT and DMA with the second half."""
                    if "o" in ABL:
                        return
                    ti = qc * 4 + sub
                    o_ps = shr.tile([128, 512], F32, name="shr_ps",
                                    tag="shr")
                    for hc in range(2):
                        lw = (pt_const[:, 0:128] if "c" in ABL
                              else ctxT[hc][:, ts(ti, 128)])
                        nc.tensor.matmul(
                            o_ps, lhsT=lw,
                            rhs=wo_sb[:, hc, ts(oc, 512)],
                            start=(hc == 0), stop=(hc == 1))
                    if op_dma == "psum":
                        if "d" not in ABL:
                            nc.sync.dma_start(
                                out[ti * 128:(ti + 1) * 128,
                                    ds(oc * 512, 512)], o_ps)
                        return
                    if oc == 0:
                        o_state[ti] = outp.tile([128, HID], BF16,
                                                name="o_sb")
                    o_sb = o_state.pop(ti) if oc == 1 else o_state[ti]
                    op_evac = os.environ.get("OP_EVAC", "dve")
                    opch = int(os.environ.get("OPCH", "1"))
                    if (op_evac == "act" or (tail and oc == 1)
                            or (op_evac == "alt" and oc == 1)):
                        nc.scalar.activation(o_sb[:, ts(oc, 512)],
                                             o_ps, COPY)
                    else:
                        cw = 512 // opch
                        for cc in range(opch):
                            nc.vector.tensor_copy(
                                o_sb[:, ds(oc * 512 + cc * cw, cw)],
                                o_ps[:, ds(cc * cw, cw)])
                    if oc == 1 and "d" not in ABL:
                        nc.sync.dma_start(out[ts(ti, 128)], o_sb)

                # ---------- exp-paced quantum schedule ----------
                def seg(sn):
                    return (sn % 2, sn // 2)    # (pr, qc)

                # PV sub-chain split points (3 sub-units per column)
                c1 = (NKT + 2) // 3
                c2 = c1 + (NKT + 1) // 3
                pv_cuts = [(0, c1), (c1, c2), (c2, NKT)]

                QUANT = int(os.environ.get("QUANT", "1400"))
                nchunk = int(os.environ.get("NCHUNK", "2"))
                pvgran = int(os.environ.get("PVGRAN", "3"))

                def build_queue(par_next):
                    """Filler units (thunk, est_ns, n_shr_allocs)
                    projecting K/Q/V for the NEXT rep; ordering keeps
                    deadlines loose."""
                    jobs = kq_jobs()
                    kj = [j for j in jobs if j[0] == "k"]
                    qj = [j for j in jobs if j[0] == "q"]
                    units = []
                    for j in kj:
                        units.append((lambda jj=j:
                                      emit_kq_job(par_next, 1, jj),
                                      1720, 1))
                    units.append((lambda jj=qj[0]:
                                  emit_kq_job(par_next, 1, jj), 1720, 1))
                    for kt in range(NKT):
                        units.append(
                            (lambda t=kt: emit_v(par_next, t), 860, 1))
                    for j in qj[1:]:
                        units.append((lambda jj=j:
                                      emit_kq_job(par_next, 1, jj),
                                      1720, 1))
                    for j in kj + qj:
                        units.append((lambda jj=j:
                                      emit_kq_job(par_next, 0, jj),
                                      1720, 1))
                    return units

                for rep in range(reps):
                    par = rep % 2
                    if rep == 0:
                        # prologue: this rep's own projections (psS is
                        # free before the S stream starts), then K1 +
                        # Q1-qc0 on shr; V rides the first fillers
                        jobs = kq_jobs()
                        for j in jobs:
                            emit_kq_job(par, 0, j, pre=True)
                        kj = [j for j in jobs if j[0] == "k"]
                        qj = [j for j in jobs if j[0] == "q"]
                        for j in kj + qj[:1]:
                            emit_kq_job(par, 1, j)
                        queue = [(lambda t=kt: emit_v(par, t), 860, 1)
                                 for kt in range(NKT)]
                        queue += [(lambda jj=j: emit_kq_job(par, 1, jj),
                                   1720, 1) for j in qj[1:]]
                    else:
                        queue = []
                    if rep + 1 < reps:
                        queue += build_queue((rep + 1) % 2)
                    qi = 0
                    credit = 0.0

                    for k in range(8):
                        pinned = {}
                        if k >= 1:
                            pr1, qc1 = seg(k - 1)
                            if pvgran == 1:
                                for i2 in range(NKT):
                                    for cl in range(2):
                                        pinned.setdefault(i2, []).append(
                                            (lambda a=i2, c=cl:
                                             emit_pv_sub(par, pr1, qc1,
                                                         c, a, a + 1),
                                             220, 0))
                                pinned.setdefault(NKT - 1, []).append(
                                    (lambda p=pr1, q=qc1:
                                     (emit_norm_col(p, q, 0, nchunk),
                                      emit_norm_col(p, q, 1, nchunk),
                                      finish_seg(p, q)), 0, 0))
                            else:
                                # PVPIN=1 spreads the six PV sub-chains
                                # across the whole slot (avoids stacking
                                # on the outproj positions 1,3,5) so
                                # per-quantum PE load is even
                                if int(os.environ.get("PVPIN", "1")):
                                    posA, posB = (0, 2, 4), (5, 6, 7)
                                    nA, nB = 4, min(7, NKT - 1)
                                else:
                                    posA, posB = (0, 1, 2), (3, 4, 5)
                                    nA, nB = 2, 5
                                for n, (lo, hi) in enumerate(pv_cuts):
                                    pinned.setdefault(posA[n], []).append(
                                        (lambda a=lo, b=hi:
                                         emit_pv_sub(par, pr1, qc1, 0, a, b),
                                         220 * (hi - lo), 0))
                                    pinned.setdefault(posB[n], []).append(
                                        (lambda a=lo, b=hi:
                                         emit_pv_sub(par, pr1, qc1, 1, a, b),
                                         220 * (hi - lo), 0))
                                pinned.setdefault(nA, []).append(
                                    (lambda: emit_norm_col(pr1, qc1, 0,
                                                           nchunk),
                                     0, 0))
                                pinned.setdefault(nB, []).append(
                                    (lambda p=pr1, q=qc1:
                                     (emit_norm_col(p, q, 1, nchunk),
                                      finish_seg(p, q)), 0, 0))
                        # outproj(qc) halves spread over slots 2qc+3 and
                        # 2qc+4 at spaced positions: one shr slot each,
                        # alternating evac engines (qc2 packs into slot
                        # 7; qc3 is the tail)
                        if k >= 3:
                            opp = tuple(
                                min(int(x), NKT - 1) for x in os.environ.get(
                                    "OPPIN", "1357"))
                            oqc = (k - 3) // 2
                            if k == 7:
                                places = [(2, p // 2, p % 2, pos)
                                          for p, pos in zip(
                                              range(4), opp)]
                            else:
                                base = 0 if k % 2 == 1 else 2
                                places = [(oqc, base + p // 2, p % 2, pos)
                                          for p, pos in zip(
                                              range(4), opp)]
                            for q, sub, oc, pos in places:
                                pinned.setdefault(
                                    min(pos, NKT - 1), []).append(
                                    (lambda qq=q, s=sub, o=oc:
                                     emit_outproj_half(qq, s, o),
                                     440, 1))
                        for i in range(NKT):
                            emit_s(par, *seg(k), i)
                            credit += QUANT - 230
                            allocs = 0
                            for fn, cost, na in pinned.get(i, ()):
                                fn()
                                credit -= cost
                                allocs += na
                            # at most one shr-slot allocation per exp
                            # quantum: the 2-deep rotation then never
                            # waits on a just-queued evacuation
                            while (credit > 0 and qi < len(queue)
                                   and allocs + queue[qi][2] <= 1):
                                fn, cost, na = queue[qi]
                                qi += 1
                                fn()
                                credit -= cost
                                allocs += na

                    # dense tail: last segment's PV/norm + outproj(3),
                    # evacs split DVE/ACT; then drain leftover fillers
                    pr1, qc1 = seg(7)
                    for lo, hi in pv_cuts:
                        emit_pv_sub(par, pr1, qc1, 0, lo, hi)
                    emit_norm_col(pr1, qc1, 0, chunks=4)
                    for lo, hi in pv_cuts:
                        emit_pv_sub(par, pr1, qc1, 1, lo, hi)
                    emit_norm_col(pr1, qc1, 1, chunks=4)
                    finish_seg(pr1, qc1)
                    for qq, sub in ((2, 2), (2, 3), (3, 0), (3, 1),
                                    (3, 2), (3, 3)):
                        for oc in range(2):
                            # rep tail: ACT is idle after the last exp --
                            # split the evacuation drain across engines
                            emit_outproj_half(qq, sub, oc, tail=True)
                    while qi < len(queue):
                        fn, cost, na = queue[qi]
                        qi += 1
                        fn()

    nc.compile()
    return nc


_NC = None


def shard_inputs(x, mask, qkv_w, qkv_b, out_w):
    bf = ml_dtypes.bfloat16
    in_maps = []
    for c in range(N_CORES):
        b, g = c // 4, c % 4
        hs = slice(DH * g, DH * (g + 1))
        xTc = np.ascontiguousarray(x[b].T).astype(bf)
        idx = np.where(mask[b] != 0)[0]
        assert len(idx) <= KC, f"unmasked {len(idx)} > KC={KC}"
        xk = np.zeros((KC, HID), np.float32)
        xk[:len(idx)] = x[b][idx]
        xTkc = np.ascontiguousarray(xk.T).astype(bf)
        ones = np.zeros(KC, np.float32)
        ones[:len(idx)] = 1.0
        onekt = ones.reshape(NKT, 128).T           # [key, kt]
        vone64 = np.ascontiguousarray(
            np.repeat(onekt[:, :, None], 64, axis=2)).astype(bf)
        qb = qkv_b[hs]
        kb = qkv_b[1024 + DH * g:1024 + DH * (g + 1)]
        vb = qkv_b[2048 + DH * g:2048 + DH * (g + 1)]
        qkvbT = np.stack([qb[0:128], qb[128:256],
                          kb[0:128], kb[128:256]], axis=1).astype(np.float32)
        vbias = np.ascontiguousarray(
            onekt[:, :, None] * vb[None, None, :]).astype(bf)
        wq = np.ascontiguousarray(qkv_w[hs, :].T).astype(bf)
        wk = np.ascontiguousarray(qkv_w[1024 + DH * g:1024 + DH * (g + 1), :].T
                                  ).astype(bf)
        wv = np.ascontiguousarray(qkv_w[2048 + DH * g:2048 + DH * (g + 1), :].T
                                  ).astype(bf)
        wo = np.ascontiguousarray(out_w[:, hs].T).astype(bf)
        in_maps.append({"xT": xTc, "xTk": xTkc, "wqT": wq, "wkT": wk,
                        "wvT": wv, "woT": wo, "qkvbT": qkvbT,
                        "vbias": vbias, "vone64": vone64})
    return in_maps


def run(in_maps, **kwargs):
    global _NC
    if _NC is None:
        _NC = build_program()
    return bass_utils.run_bass_kernel_spmd(
        _NC, in_maps, core_ids=list(range(N_CORES)), **kwargs)


def kernel(x, mask, qkv_w, qkv_b, out_w, out_b):
    global KC, NKT, _NC
    x = np.asarray(x)
    mask = np.asarray(mask)
    need = int(np.max(np.sum(mask != 0, axis=1)))
    kc = max(128, ((need + 127) // 128) * 128)
    if kc != KC:
        KC, NKT = kc, kc // 128
        _NC = None
    qkv_w = np.asarray(qkv_w)
    qkv_b = np.asarray(qkv_b)
    out_w = np.asarray(out_w)
    out_b = np.asarray(out_b)
    in_maps = shard_inputs(x, mask, qkv_w, qkv_b, out_w)
    res = run(in_maps)
    parts = [r["out"] for r in res.results]
    full = np.empty((2, S, HID), np.float32)
    for b in range(2):
        acc = parts[4 * b].astype(np.float32)
        for g in range(1, 4):
            acc = acc + parts[4 * b + g].astype(np.float32)
        full[b] = acc + out_b[None, :]
    return full
